# revision 26
# baseline (speedup 1.0000x reference)
"""Distributed causal multi-head attention forward for one TRN2 chip (8 NeuronCores).

Problem (nn_Attention): B=2, S=2048, d_model=1024, 16 heads x 64.
    attn_in = x + pos_embed
    q = attn_in @ W_Q + b_Q ; k = attn_in @ W_K + b_K ; v = x @ W_V + b_V
    out = softmax(causal(q k^T / sqrt(64))) v @ W_O + b_O

Sharding: data-parallel over batch (2 groups of 4 cores), tensor-parallel over
heads inside each group (4 heads per core).  Each core computes the partial
output  sum_h z_h @ W_O_h  for its heads per 512-row block, then a
ReduceScatter(add, bf16) over the 4-core group leaves each core with 128 rows
of the fully-summed block.  The host reassembles the full [B, S, D] tensor
from the striped row shards (pure gather/indexing, no arithmetic).

Layout/perf notes:
  * All inputs arrive in bf16 (host-side cast): halves HBM traffic vs fp32;
    all matmuls accumulate in fp32 PSUM (~0.3-0.6% final error vs the 2e-2
    gate).  Weights come host-pre-chunked so each weight tensor is ONE
    contiguous [128, ...] DMA (descriptor-gen on the DGE queues costs ~625ns
    per DMA instruction, so DMA count matters more than DMA bytes).
  * x/pos are loaded as full-sequence rows ([128, 2048] per d_model chunk, 8
    DMAs each), split across both DGE queues (SP + Activation).
  * Scores are built transposed (keys on partitions) so softmax-exp feeds
    P@V directly.  Matmul cost is (out free size) cycles, so P@V runs
    "z^T-wise": out [q=128, d_head+1] with a 65-wide moving dim.
  * Softmax denominator: ones-column appended to V; normalization is a
    per-partition reciprocal + tensor_scalar on DVE.  z^T -> z transposes
    for W_O are dma_start_transpose (XBAR): zero PE cycles.
  * Emission is software-pipelined and CIRCULAR across reps: B(J) injects
    the QKV projections of block J+1 (wrapping into the next rep) and the
    W_O + ReduceScatter of block J-1 between score chunks, so the in-order
    PE queue stays fed while the Activation engine (exp) catches up.
"""

import math

import numpy as np

import concourse.bass as bass  # noqa: F401  (bass must import before bacc)
import concourse.mybir as mybir
from concourse import bacc, tile
from concourse.bass_utils import run_bass_kernel_spmd

B, S, D = 2, 2048, 1024
NH, DH = 16, 64
N_CORES = 8
GPC = 4                      # cores per batch group
HPC = NH // GPC              # heads per core
QB = 512                     # query-block rows
NJ = S // QB                 # query blocks
KCH = 128                    # key chunk (= row tile)
DCH = D // 128               # d_model chunks
RG = [[0, 1, 2, 3], [4, 5, 6, 7]]
SCALE = 1.0 / float(np.sqrt(DH))

F32 = mybir.dt.float32


class _ActCopy:
    """Adapter: .tensor_copy on the Activation engine (activation Copy)."""
    def __init__(self, nc):
        self._nc = nc

    def tensor_copy(self, out, in_):
        self._nc.scalar.copy(out, in_)
BF16 = mybir.dt.bfloat16
EXP = mybir.ActivationFunctionType.Exp
ADD = mybir.AluOpType.add
MUL = mybir.AluOpType.mult


def build_nc(reps: int = 1, collective: bool = True, bias: bool = True,
             rs_f32: bool = False):
    """Build the per-core Bass graph.  `reps` repeats the whole computation
    (used only for wall-clock timing calibration; grading uses reps=1)."""
    nc = bacc.Bacc("TRN2", target_bir_lowering=False, debug=False,
                   num_devices=N_CORES)

    xT = nc.dram_tensor("xT", [D, S], BF16, kind="ExternalInput").ap()
    posT = nc.dram_tensor("posT", [D, S], BF16, kind="ExternalInput").ap()
    # host-pre-chunked: [128, kc, 256] flattened
    wqp = nc.dram_tensor("wqp", [128, DCH * HPC * DH], BF16,
                         kind="ExternalInput").ap()
    wkp = nc.dram_tensor("wkp", [128, DCH * HPC * DH], BF16,
                         kind="ExternalInput").ap()
    wvp = nc.dram_tensor("wvp", [128, DCH * HPC * DH], BF16,
                         kind="ExternalInput").ap()
    wop = nc.dram_tensor("wop", [128, 2 * D], BF16, kind="ExternalInput").ap()
    bqT = nc.dram_tensor("bqT", [KCH, 2], BF16, kind="ExternalInput").ap()
    bkT = nc.dram_tensor("bkT", [KCH, 2], BF16, kind="ExternalInput").ap()
    bv = nc.dram_tensor("bv", [1, HPC * DH], BF16, kind="ExternalInput").ap()
    bo = nc.dram_tensor("bo", [1, D], BF16, kind="ExternalInput").ap()
    masks = nc.dram_tensor("masks", [KCH, 2 * KCH], BF16,
                           kind="ExternalInput").ap()
    rdt = F32 if rs_f32 else BF16
    out_ext = nc.dram_tensor("out", [S // GPC, D], rdt,
                             kind="ExternalOutput").ap()

    act_copy = _ActCopy(nc)
    with tile.TileContext(nc) as tc:
        with tc.tile_pool(name="wp", bufs=1) as wp, \
             tc.tile_pool(name="qkv", bufs=1) as qp, \
             tc.tile_pool(name="xfp", bufs=9) as xfp, \
             tc.tile_pool(name="posp", bufs=2) as posp, \
             tc.tile_pool(name="xpp", bufs=9) as xpp, \
             tc.tile_pool(name="p2p", bufs=16) as p2p, \
             tc.tile_pool(name="rsp", bufs=4) as rsp, \
             tc.tile_pool(name="ztsb", bufs=16) as ztsbp, \
             tc.tile_pool(name="ztp", bufs=16) as ztpp, \
             tc.tile_pool(name="osb", bufs=3) as osbp, \
             tc.tile_pool(name="psS", bufs=2, space="PSUM") as psS, \
             tc.tile_pool(name="psZT", bufs=2, space="PSUM") as psZT, \
             tc.tile_pool(name="psA", bufs=2, space="PSUM") as psA, \
             tc.tile_pool(name="dram", bufs=2, space="DRAM") as dp:

            # ---------- persistent weight tiles ----------
            wq_t = wp.tile([128, DCH, HPC * DH], BF16, tag="wq")
            wk_t = wp.tile([128, DCH, HPC * DH], BF16, tag="wk")
            wv_t = wp.tile([128, DCH, HPC * DH], BF16, tag="wv")
            wo_t = wp.tile([128, 2, D], BF16, tag="wo")
            tri_m = wp.tile([KCH, 2, KCH], BF16, tag="tri_m")
            bqT_t = wp.tile([KCH, 2], BF16, tag="bqT")
            bkT_t = wp.tile([KCH, 2], BF16, tag="bkT")
            bv_t = wp.tile([1, HPC * DH], BF16, tag="bv")
            bo_t = wp.tile([1, D], BF16, tag="bo")

            def emit_weight_dmas():
                # wq first (first consumer); split across the two DGE queues
                nc.sync.dma_start(wq_t[:], wqp[:, :])
                nc.scalar.dma_start(wk_t[:], wkp[:, :])
                nc.sync.dma_start(wv_t[:], wvp[:, :])
                nc.scalar.dma_start(wo_t[:], wop[:, :])
                nc.scalar.dma_start(tri_m[:, :, :], masks[:, :])
                nc.scalar.dma_start(bqT_t[:], bqT[:, :])
                nc.scalar.dma_start(bkT_t[:], bkT[:, :])
                nc.scalar.dma_start(bv_t[:], bv[:, :])
                nc.scalar.dma_start(bo_t[:], bo[:, :])

            if bias:
                emit_weight_dmas()
                ones = wp.tile([1, KCH], BF16, tag="ones")
                nc.vector.memset(ones[:], 1.0)
                bv_ps = psA.tile([128, HPC, DH], F32, tag="a_ps")
                nc.tensor.matmul(bv_ps[:], ones[0:1, :], bv_t[0:1, :],
                                 start=True, stop=True)
                bv_bc = wp.tile([128, HPC, DH], BF16, tag="bv_bc")
                nc.vector.tensor_copy(bv_bc[:], bv_ps[:])
                bo_bc = wp.tile([128, D], BF16, tag="bo_bc")
                for ms in range(2):
                    bo_ps = psA.tile([128, 512], F32, tag="a_ps")
                    nc.tensor.matmul(bo_ps[:], ones[0:1, :],
                                     bo_t[0:1, 512 * ms:512 * (ms + 1)],
                                     start=True, stop=True)
                    nc.vector.tensor_copy(
                        bo_bc[:, 512 * ms:512 * (ms + 1)], bo_ps[:])
            weights_loaded = bool(bias)

            # persistent per-rep activations, double-buffered by rep parity
            # so the next rep's QKV can overlap this rep's attention tail
            npar = 2 if reps > 1 else 1
            qT_par, kT_par, va_par = [], [], []
            for par in range(npar):
                qT, kT = [], []
                for p in range(2):
                    t_q = qp.tile([128, S], BF16, tag=f"qT{par}{p}",
                                  name="t_q")
                    qT.append(t_q)
                    t_k = qp.tile([128, S], BF16, tag=f"kT{par}{p}",
                                  name="t_k")
                    kT.append(t_k)
                v_aug = []
                for rt in range(S // KCH):
                    t_v = qp.tile([128, HPC, DH + 1], BF16,
                                  tag=f"va{par}_{rt}", name="t_v")
                    nc.vector.memset(t_v[:, :, DH:DH + 1], 1.0)
                    v_aug.append(t_v)
                qT_par.append(qT)
                kT_par.append(kT)
                va_par.append(v_aug)

            # rolling per-block state (overwritten every rep)
            x_par = {}         # parity -> list of x row tiles
            xp_par = {}        # parity -> list of x+pos row tiles
            zts = {}           # (J, hp, qsub) -> zt_sb tile
            prt = [None] * NJ
            tz_stash = {}

            def emit_producers(par):
                """DMA full-sequence x/pos rows + adds for one whole rep."""
                xs, xps = [], []
                for kc in range(DCH):
                    ksl = slice(128 * kc, 128 * (kc + 1))
                    t_xc = xfp.tile([128, S], BF16, tag="xc", name="xc")
                    nc.sync.dma_start(t_xc[:], xT[ksl, :])
                    t_pos = posp.tile([128, S], BF16, tag="pos", name="pos")
                    nc.scalar.dma_start(t_pos[:], posT[ksl, :])
                    t_xp = xpp.tile([128, S], BF16, tag="xp", name="xp")
                    nc.gpsimd.tensor_add(t_xp[:], t_xc[:], t_pos[:])
                    xs.append(t_xc)
                    xps.append(t_xp)
                x_par[par] = xs
                xp_par[par] = xps

            def qk_group(J, par, dst, w_t, b_t, p, ceng=None):
                xp_t = xp_par[par]
                jsl = slice(QB * J, QB * (J + 1))
                psl = slice(128 * p, 128 * (p + 1))
                acc = psA.tile([128, QB], F32, tag="a_ps")
                for kc in range(DCH):
                    nc.tensor.matmul(acc[:], w_t[:, kc, psl],
                                     xp_t[kc][:, jsl],
                                     start=(kc == 0), stop=(kc == DCH - 1))
                if bias:
                    nc.vector.tensor_scalar(
                        dst[p][:, jsl], acc[:], b_t[:, p:p + 1], None, ADD)
                else:
                    (ceng or nc.vector).tensor_copy(dst[p][:, jsl], acc[:])

            def v_group(J, par, r, ceng=None):
                v_aug = va_par[par]
                x_t = x_par[par]
                rt = 4 * J + r
                rsl = slice(QB * J + 128 * r, QB * J + 128 * (r + 1))
                vacc = psA.tile([128, HPC, DH], F32, tag="a_ps")
                for kc in range(DCH):
                    nc.tensor.matmul(vacc[:], x_t[kc][:, rsl], wv_t[:, kc, :],
                                     start=(kc == 0), stop=(kc == DCH - 1))
                if bias:
                    nc.vector.tensor_tensor(
                        v_aug[rt][:, :, 0:DH], vacc[:], bv_bc[:], ADD)
                else:
                    (ceng or nc.vector).tensor_copy(
                        v_aug[rt][:, :, 0:DH], vacc[:])

            def qkv_groups(J, par, ceng=None):
                gs = []
                for dst, w_t, b_t in ((qT_par[par], wq_t, bqT_t),
                                      (kT_par[par], wk_t, bkT_t)):
                    for p in range(2):
                        gs.append(lambda J=J, par=par, dst=dst, w_t=w_t,
                                  b_t=b_t, p=p:
                                  qk_group(J, par, dst, w_t, b_t, p, ceng))
                for r in range(4):
                    gs.append(lambda J=J, par=par, r=r:
                              v_group(J, par, r, ceng))
                return gs

            def wo_qsub(J, qsub, ceng=None):
                """W_O for one 128-row qsub of block J: 2 XBAR transposes +
                2 psum groups + copies + one merged prt row-block DMA."""
                for hp in range(2):
                    t_tz = ztpp.tile([128, 128], BF16, tag="ztp", name="tz")
                    eng = nc.sync if hp == 0 else nc.scalar
                    eng.dma_start_transpose(t_tz[:], zts[(J, hp, qsub)][:])
                    tz_stash[(J, qsub, hp)] = t_tz
                o_sb = osbp.tile([128, D], rdt, tag="o_sb")
                for n2 in range(2):
                    msl = slice(512 * n2, 512 * (n2 + 1))
                    oacc = psA.tile([128, 512], F32, tag="a_ps")
                    for hp in range(2):
                        nc.tensor.matmul(oacc[:], tz_stash[(J, qsub, hp)][:],
                                         wo_t[:, hp, msl],
                                         start=(hp == 0), stop=(hp == 1))
                    if bias:
                        # host pre-scales b_O by 1/GPC: every core adds
                        # bias/GPC so the ReduceScatter sum is exact.
                        nc.vector.tensor_tensor(o_sb[:, msl], oacc[:],
                                                bo_bc[:, msl], ADD)
                    else:
                        (ceng or nc.vector).tensor_copy(o_sb[:, msl], oacc[:])
                eng = nc.sync if qsub % 2 == 0 else nc.scalar
                eng.dma_start(
                    prt[J][128 * qsub:128 * (qsub + 1), :], o_sb[:])

            def wo_rs_out(J):
                rs = dp.tile([QB // GPC, D], rdt, tag="rs", name="rs")
                if collective:
                    nc.gpsimd.collective_compute(
                        "ReduceScatter", mybir.AluOpType.add,
                        replica_groups=RG,
                        ins=[prt[J][:].opt()], outs=[rs[:].opt()])
                else:
                    nc.sync.dma_start(rs[:], prt[J][0:128, :])
                nc.scalar.dma_start(
                    out_ext[128 * J:128 * (J + 1), :], rs[:])

            def emit_S_exp(J, par, hp, c):
                qT, kT = qT_par[par], kT_par[par]
                dlt = c - 4 * J
                w0 = 128 * dlt if dlt >= 0 else 0
                csl = slice(KCH * c, KCH * (c + 1))
                qsl = slice(QB * J + w0, QB * (J + 1))
                lo, hi = slice(0, 64), slice(64, 128)
                s2 = psS.tile([KCH, 2, QB], F32, tag="s2")
                nc.tensor.matmul(s2[:, 0, w0:QB], kT[hp][lo, csl],
                                 qT[hp][lo, qsl], start=True, stop=True)
                nc.tensor.matmul(s2[:, 1, w0:QB], kT[hp][hi, csl],
                                 qT[hp][hi, qsl], start=True, stop=True)
                p2 = p2p.tile([KCH, 2, QB], BF16, tag="p2")
                nc.scalar.activation(p2[:, :, w0:QB], s2[:, :, w0:QB],
                                     EXP, scale=SCALE)
                if dlt >= 0:
                    nc.vector.tensor_mul(p2[:, :, w0:w0 + KCH],
                                         p2[:, :, w0:w0 + KCH], tri_m[:])
                return p2

            def emit_PV(J, par, hp, p2s, after_qsub=None):
                v_aug = va_par[par]
                for qsub in range(4):
                    zt = psZT.tile([KCH, 2, DH + 1], F32, tag="zt")
                    nch_q = 4 * J + qsub + 1
                    qo = 128 * qsub
                    for hh in range(2):
                        h = 2 * hp + hh
                        for c in range(nch_q):
                            nc.tensor.matmul(
                                zt[:, hh, :],
                                p2s[c][:, hh, qo:qo + 128],
                                v_aug[c][:, h, :],
                                start=(c == 0), stop=(c == nch_q - 1))
                    rsb = rsp.tile([KCH, 2, 1], F32, tag="rsb")
                    nc.vector.reciprocal(rsb[:], zt[:, :, DH:DH + 1])
                    zt_sb = ztsbp.tile([KCH, 2, DH], BF16, tag="ztsb")
                    for hh in range(2):
                        nc.vector.tensor_scalar(
                            zt_sb[:, hh, :], zt[:, hh, 0:DH],
                            rsb[:, hh, :], None, MUL)
                    zts[(J, hp, qsub)] = zt_sb
                    if after_qsub is not None:
                        after_qsub(qsub)

            for _rep in range(reps):
                par = _rep % npar
                for jb in range(NJ):
                    J = jb
                    nch = 4 * (J + 1)
                    prt[J] = dp.tile([QB, D], rdt, tag="prt", name="prt")
                    if _rep == 0 and J == 0:
                        # bootstrap: weights + rep-0 x/pos + QKV(0)
                        if not weights_loaded:
                            nc.sync.dma_start(wq_t[:], wqp[:, :])
                        emit_producers(par)
                        if not weights_loaded:
                            nc.scalar.dma_start(wk_t[:], wkp[:, :])
                            nc.sync.dma_start(wv_t[:], wvp[:, :])
                            nc.scalar.dma_start(wo_t[:], wop[:, :])
                            nc.scalar.dma_start(tri_m[:, :, :], masks[:, :])
                            nc.scalar.dma_start(bqT_t[:], bqT[:, :])
                            nc.scalar.dma_start(bkT_t[:], bkT[:, :])
                            nc.scalar.dma_start(bv_t[:], bv[:, :])
                            nc.scalar.dma_start(bo_t[:], bo[:, :])
                            weights_loaded = True
                        for g in qkv_groups(0, par):
                            g()

                    # filler: PE work injectable between score chunks
                    filler = []
                    if J >= 1:
                        Jw = J - 1
                        woc = None
                        for q in range(4):
                            filler.append(lambda Jw=Jw, q=q, woc=woc:
                                          wo_qsub(Jw, q, woc))
                        filler.append(lambda Jw=Jw: wo_rs_out(Jw))
                    elif _rep >= 1:
                        for q in range(4):
                            filler.append(lambda q=q: wo_qsub(3, q))
                        filler.append(lambda: wo_rs_out(3))
                    if J + 1 < NJ:
                        qc = None
                        filler += qkv_groups(J + 1, par, qc)
                    elif _rep + 1 < reps:
                        emit_producers((_rep + 1) % npar)
                        filler += qkv_groups(0, (_rep + 1) % npar)

                    last_block = (_rep == reps - 1 and J == NJ - 1)
                    slots = 2 * nch
                    for hp in range(2):
                        p2s = []
                        for c in range(nch):
                            p2s.append(emit_S_exp(J, par, hp, c))
                            take = (math.ceil(len(filler) / slots)
                                    if slots > 1 else len(filler))
                            for _ in range(take):
                                filler.pop(0)()
                            slots -= 1
                        if last_block and hp == 1:
                            # tail: fold W_O(3) into PV(3, hp1) per qsub
                            def _tail(qsub):
                                wo_qsub(3, qsub)
                                if qsub == 3:
                                    wo_rs_out(3)
                            emit_PV(J, par, hp, p2s, after_qsub=_tail)
                        else:
                            emit_PV(J, par, hp, p2s)
    nc.compile()
    return nc


def _make_masks():
    # [128, 2*128] causal triangle duplicated for the head-pair layout:
    # tri[k, j] = 1 if k <= j (the diagonal band of every 128-key chunk
    # relative to its causal column start)
    k = np.arange(KCH)[:, None]
    j = np.arange(KCH)[None, :]
    tri = (k <= j).astype(np.float32)
    return np.ascontiguousarray(np.concatenate([tri, tri], axis=1))


def _prechunk(w):
    """[1024, C] -> [128, DCH*C] with kc-major free layout."""
    c = w.shape[1]
    return np.ascontiguousarray(
        w.reshape(DCH, 128, c).transpose(1, 0, 2).reshape(128, DCH * c))


def make_in_maps(x, pos_embed, W_Q, b_Q, W_K, b_K, W_V, b_V, W_O, b_O):
    import ml_dtypes
    bf = ml_dtypes.bfloat16
    x = np.asarray(x, np.float32)
    pos_embed = np.asarray(pos_embed, np.float32)
    W_Q = np.asarray(W_Q, np.float32)
    W_K = np.asarray(W_K, np.float32)
    W_V = np.asarray(W_V, np.float32)
    W_O = np.asarray(W_O, np.float32)
    b_Q = np.asarray(b_Q, np.float32)
    b_K = np.asarray(b_K, np.float32)
    b_V = np.asarray(b_V, np.float32)
    b_O = np.asarray(b_O, np.float32)
    masks = _make_masks().astype(bf)
    in_maps = []
    for c in range(N_CORES):
        g, j = divmod(c, GPC)
        hs = slice(HPC * j, HPC * (j + 1))
        wo_pair = np.ascontiguousarray(
            W_O[hs].reshape(2, 128, D).transpose(1, 0, 2).reshape(128, 2 * D))
        in_maps.append({
            "xT": np.ascontiguousarray(x[g].T).astype(bf),
            "posT": np.ascontiguousarray(pos_embed[g].T).astype(bf),
            "wqp": _prechunk(
                W_Q[hs].transpose(1, 0, 2).reshape(D, HPC * DH)).astype(bf),
            "wkp": _prechunk(
                W_K[hs].transpose(1, 0, 2).reshape(D, HPC * DH)).astype(bf),
            "wvp": _prechunk(
                W_V[hs].transpose(1, 0, 2).reshape(D, HPC * DH)).astype(bf),
            "wop": wo_pair.astype(bf),
            "bqT": np.ascontiguousarray(
                b_Q[hs].reshape(2, KCH).T).astype(bf),
            "bkT": np.ascontiguousarray(
                b_K[hs].reshape(2, KCH).T).astype(bf),
            "bv": np.ascontiguousarray(
                b_V[hs].reshape(1, HPC * DH)).astype(bf),
            "bo": np.ascontiguousarray(
                (b_O / GPC).reshape(1, D)).astype(bf),
            "masks": masks,
        })
    return in_maps


def assemble_out(results):
    out = np.empty((B, S, D), np.float32)
    for c in range(N_CORES):
        g, j = divmod(c, GPC)
        o = results[c]["out"].astype(np.float32).reshape(NJ, 128, D)
        for J in range(NJ):
            out[g, QB * J + 128 * j:QB * J + 128 * (j + 1), :] = o[J]
    return out


_BUILT = {}


def get_built(reps: int = 1, bias: bool = True, collective: bool = True,
              rs_f32: bool = False):
    key = (reps, bias, collective, rs_f32)
    if key not in _BUILT:
        _BUILT[key] = build_nc(reps, collective=collective, bias=bias,
                               rs_f32=rs_f32)
    return _BUILT[key]


def kernel(**inputs) -> np.ndarray:
    use_bias = any(
        np.any(np.asarray(inputs[k])) for k in ("b_Q", "b_K", "b_V", "b_O"))
    nc = get_built(1, bias=bool(use_bias))
    in_maps = make_in_maps(**inputs)
    res = run_bass_kernel_spmd(nc, in_maps, list(range(N_CORES)))
    return assemble_out(res.results)


# revision 28
# speedup vs baseline: 1.6266x; 1.6266x over previous
"""Distributed causal multi-head attention forward for one TRN2 chip (8 NeuronCores).

Problem (nn_Attention): B=2, S=2048, d_model=1024, 16 heads x 64.
    attn_in = x + pos_embed
    q = attn_in @ W_Q + b_Q ; k = attn_in @ W_K + b_K ; v = x @ W_V + b_V
    out = softmax(causal(q k^T / sqrt(64))) v @ W_O + b_O

Sharding: data-parallel over batch (2 groups of 4 cores), tensor-parallel over
heads inside each group (4 heads per core).  Each core computes the partial
output  sum_h z_h @ W_O_h  for its heads per 512-row block, then a
ReduceScatter(add, bf16) over the 4-core group leaves each core with 128 rows
of the fully-summed block.  The host reassembles the full [B, S, D] tensor
from the striped row shards (pure gather/indexing, no arithmetic).

Layout/perf notes:
  * All inputs arrive in bf16 (host-side cast): halves HBM traffic vs fp32;
    all matmuls accumulate in fp32 PSUM (~0.3-0.6% final error vs the 2e-2
    gate).  Weights come host-pre-chunked so each weight tensor is ONE
    contiguous [128, ...] DMA (descriptor-gen on the DGE queues costs ~625ns
    per DMA instruction, so DMA count matters more than DMA bytes).
  * x/pos are loaded as full-sequence rows ([128, 2048] per d_model chunk, 8
    DMAs each), split across both DGE queues (SP + Activation).
  * Scores are built transposed (keys on partitions) so softmax-exp feeds
    P@V directly.  Matmul cost is (out free size) cycles, so P@V runs
    "z^T-wise": out [q=128, d_head+1] with a 65-wide moving dim.
  * Softmax denominator: ones-column appended to V; normalization is a
    per-partition reciprocal + tensor_scalar on DVE.  z^T -> z transposes
    for W_O are dma_start_transpose (XBAR): zero PE cycles.
  * Emission is software-pipelined and CIRCULAR across reps: B(J) injects
    the QKV projections of block J+1 (wrapping into the next rep) and the
    W_O + ReduceScatter of block J-1 between score chunks, so the in-order
    PE queue stays fed while the Activation engine (exp) catches up.
"""

import math

import numpy as np

import concourse.bass as bass  # noqa: F401  (bass must import before bacc)
import concourse.mybir as mybir
from concourse import bacc, tile
from concourse.bass_utils import run_bass_kernel_spmd

B, S, D = 2, 2048, 1024
NH, DH = 16, 64
N_CORES = 8
GPC = 4                      # cores per batch group
HPC = NH // GPC              # heads per core
QB = 512                     # query-block rows
NJ = S // QB                 # query blocks
KCH = 128                    # key chunk (= row tile)
DCH = D // 128               # d_model chunks
RG = [[0, 1, 2, 3], [4, 5, 6, 7]]
SCALE = 1.0 / float(np.sqrt(DH))

F32 = mybir.dt.float32


class _ActCopy:
    """Adapter: .tensor_copy on the Activation engine (activation Copy)."""
    def __init__(self, nc):
        self._nc = nc

    def tensor_copy(self, out, in_):
        self._nc.scalar.copy(out, in_)
BF16 = mybir.dt.bfloat16
EXP = mybir.ActivationFunctionType.Exp
ADD = mybir.AluOpType.add
MUL = mybir.AluOpType.mult


def build_nc(reps: int = 1, collective: bool = True, bias: bool = True,
             rs_f32: bool = False, rs_rep: bool = False, ag: bool = False):
    """Build the per-core Bass graph.  `reps` repeats the whole computation
    (used only for wall-clock timing calibration; grading uses reps=1)."""
    nc = bacc.Bacc("TRN2", target_bir_lowering=False, debug=False,
                   num_devices=N_CORES)

    xT = nc.dram_tensor("xT", [D, S], BF16, kind="ExternalInput").ap()
    posT = nc.dram_tensor("posT", [D, S], BF16, kind="ExternalInput").ap()
    # host-pre-chunked: [128, kc, 256] flattened
    wqp = nc.dram_tensor("wqp", [128, DCH * HPC * DH], BF16,
                         kind="ExternalInput").ap()
    wkp = nc.dram_tensor("wkp", [128, DCH * HPC * DH], BF16,
                         kind="ExternalInput").ap()
    wvp = nc.dram_tensor("wvp", [128, DCH * HPC * DH], BF16,
                         kind="ExternalInput").ap()
    if ag:
        wof = nc.dram_tensor("wof", [128, 8 * D], BF16,
                             kind="ExternalInput").ap()
        sel = nc.dram_tensor("sel", [128, GPC * 128], BF16,
                             kind="ExternalInput").ap()
    else:
        wop = nc.dram_tensor("wop", [128, 2 * D], BF16,
                             kind="ExternalInput").ap()
    bqT = nc.dram_tensor("bqT", [KCH, 2], BF16, kind="ExternalInput").ap()
    bkT = nc.dram_tensor("bkT", [KCH, 2], BF16, kind="ExternalInput").ap()
    bv = nc.dram_tensor("bv", [1, HPC * DH], BF16, kind="ExternalInput").ap()
    bo = nc.dram_tensor("bo", [1, D], BF16, kind="ExternalInput").ap()
    masks = nc.dram_tensor("masks", [KCH, 2 * KCH], BF16,
                           kind="ExternalInput").ap()
    rdt = F32 if rs_f32 else BF16
    if ag:
        rdt = F32
    out_ext = nc.dram_tensor("out", [S // GPC, D], rdt,
                             kind="ExternalOutput").ap()

    act_copy = _ActCopy(nc)
    with tile.TileContext(nc) as tc:
        with tc.tile_pool(name="wp", bufs=1) as wp, \
             tc.tile_pool(name="qkv", bufs=1) as qp, \
             tc.tile_pool(name="xfp", bufs=9) as xfp, \
             tc.tile_pool(name="posp", bufs=2) as posp, \
             tc.tile_pool(name="xpp", bufs=9) as xpp, \
             tc.tile_pool(name="p2p", bufs=16) as p2p, \
             tc.tile_pool(name="rsp", bufs=4) as rsp, \
             tc.tile_pool(name="ztsb", bufs=16) as ztsbp, \
             tc.tile_pool(name="ztp", bufs=16) as ztpp, \
             tc.tile_pool(name="osb", bufs=3) as osbp, \
             tc.tile_pool(name="psS", bufs=2, space="PSUM") as psS, \
             tc.tile_pool(name="psZT", bufs=2, space="PSUM") as psZT, \
             tc.tile_pool(name="psA", bufs=2, space="PSUM") as psA, \
             tc.tile_pool(name="dram", bufs=2, space="DRAM") as dp:

            # ---------- persistent weight tiles ----------
            wq_t = wp.tile([128, DCH, HPC * DH], BF16, tag="wq")
            wk_t = wp.tile([128, DCH, HPC * DH], BF16, tag="wk")
            wv_t = wp.tile([128, DCH, HPC * DH], BF16, tag="wv")
            if ag:
                wo_t = wp.tile([128, 8, D], BF16, tag="wo")
                sel_t = wp.tile([128, GPC, 128], BF16, tag="sel")
            else:
                wo_t = wp.tile([128, 2, D], BF16, tag="wo")
            tri_m = wp.tile([KCH, 2, KCH], BF16, tag="tri_m")
            bqT_t = wp.tile([KCH, 2], BF16, tag="bqT")
            bkT_t = wp.tile([KCH, 2], BF16, tag="bkT")
            bv_t = wp.tile([1, HPC * DH], BF16, tag="bv")
            bo_t = wp.tile([1, D], BF16, tag="bo")

            def emit_weight_dmas():
                # wq first (first consumer); split across the two DGE queues
                nc.sync.dma_start(wq_t[:], wqp[:, :])
                nc.scalar.dma_start(wk_t[:], wkp[:, :])
                nc.sync.dma_start(wv_t[:], wvp[:, :])
                if ag:
                    nc.scalar.dma_start(wo_t[:], wof[:, :])
                    nc.scalar.dma_start(sel_t[:], sel[:, :])
                else:
                    nc.scalar.dma_start(wo_t[:], wop[:, :])
                nc.scalar.dma_start(tri_m[:, :, :], masks[:, :])
                nc.scalar.dma_start(bqT_t[:], bqT[:, :])
                nc.scalar.dma_start(bkT_t[:], bkT[:, :])
                nc.scalar.dma_start(bv_t[:], bv[:, :])
                nc.scalar.dma_start(bo_t[:], bo[:, :])

            if bias:
                emit_weight_dmas()
                ones = wp.tile([1, KCH], BF16, tag="ones")
                nc.vector.memset(ones[:], 1.0)
                bv_ps = psA.tile([128, HPC, DH], F32, tag="a_ps")
                nc.tensor.matmul(bv_ps[:], ones[0:1, :], bv_t[0:1, :],
                                 start=True, stop=True)
                bv_bc = wp.tile([128, HPC, DH], BF16, tag="bv_bc")
                nc.vector.tensor_copy(bv_bc[:], bv_ps[:])
                bo_bc = wp.tile([128, D], BF16, tag="bo_bc")
                for ms in range(2):
                    bo_ps = psA.tile([128, 512], F32, tag="a_ps")
                    nc.tensor.matmul(bo_ps[:], ones[0:1, :],
                                     bo_t[0:1, 512 * ms:512 * (ms + 1)],
                                     start=True, stop=True)
                    nc.vector.tensor_copy(
                        bo_bc[:, 512 * ms:512 * (ms + 1)], bo_ps[:])
            weights_loaded = bool(bias)

            # persistent per-rep activations, double-buffered by rep parity
            # so the next rep's QKV can overlap this rep's attention tail
            npar = 2 if reps > 1 else 1
            qT_par, kT_par, va_par = [], [], []
            for par in range(npar):
                qT, kT = [], []
                for p in range(2):
                    t_q = qp.tile([128, S], BF16, tag=f"qT{par}{p}",
                                  name="t_q")
                    qT.append(t_q)
                    t_k = qp.tile([128, S], BF16, tag=f"kT{par}{p}",
                                  name="t_k")
                    kT.append(t_k)
                v_aug = []
                for rt in range(S // KCH):
                    t_v = qp.tile([128, HPC, DH + 1], BF16,
                                  tag=f"va{par}_{rt}", name="t_v")
                    nc.vector.memset(t_v[:, :, DH:DH + 1], 1.0)
                    v_aug.append(t_v)
                qT_par.append(qT)
                kT_par.append(kT)
                va_par.append(v_aug)

            # rolling per-block state (overwritten every rep)
            x_par = {}         # parity -> list of x row tiles
            xp_par = {}        # parity -> list of x+pos row tiles
            zts = {}           # (J, hp, qsub) -> zt_sb tile
            prt = [None] * NJ
            tz_stash = {}

            def emit_producers(par):
                """DMA full-sequence x/pos rows + adds for one whole rep."""
                xs, xps = [], []
                for kc in range(DCH):
                    ksl = slice(128 * kc, 128 * (kc + 1))
                    t_xc = xfp.tile([128, S], BF16, tag="xc", name="xc")
                    nc.sync.dma_start(t_xc[:], xT[ksl, :])
                    t_pos = posp.tile([128, S], BF16, tag="pos", name="pos")
                    nc.scalar.dma_start(t_pos[:], posT[ksl, :])
                    t_xp = xpp.tile([128, S], BF16, tag="xp", name="xp")
                    if ag:
                        nc.vector.tensor_add(t_xp[:], t_xc[:], t_pos[:])
                    else:
                        nc.gpsimd.tensor_add(t_xp[:], t_xc[:], t_pos[:])
                    xs.append(t_xc)
                    xps.append(t_xp)
                x_par[par] = xs
                xp_par[par] = xps

            def qk_group(J, par, dst, w_t, b_t, p, ceng=None):
                xp_t = xp_par[par]
                jsl = slice(QB * J, QB * (J + 1))
                psl = slice(128 * p, 128 * (p + 1))
                acc = psA.tile([128, QB], F32, tag="a_ps")
                for kc in range(DCH):
                    nc.tensor.matmul(acc[:], w_t[:, kc, psl],
                                     xp_t[kc][:, jsl],
                                     start=(kc == 0), stop=(kc == DCH - 1))
                if bias:
                    nc.vector.tensor_scalar(
                        dst[p][:, jsl], acc[:], b_t[:, p:p + 1], None, ADD)
                else:
                    (ceng or nc.vector).tensor_copy(dst[p][:, jsl], acc[:])

            def v_group(J, par, r, ceng=None):
                v_aug = va_par[par]
                x_t = x_par[par]
                rt = 4 * J + r
                rsl = slice(QB * J + 128 * r, QB * J + 128 * (r + 1))
                vacc = psA.tile([128, HPC, DH], F32, tag="a_ps")
                for kc in range(DCH):
                    nc.tensor.matmul(vacc[:], x_t[kc][:, rsl], wv_t[:, kc, :],
                                     start=(kc == 0), stop=(kc == DCH - 1))
                if bias:
                    nc.vector.tensor_tensor(
                        v_aug[rt][:, :, 0:DH], vacc[:], bv_bc[:], ADD)
                else:
                    (ceng or nc.vector).tensor_copy(
                        v_aug[rt][:, :, 0:DH], vacc[:])

            def qkv_groups(J, par, ceng=None):
                gs = []
                for dst, w_t, b_t in ((qT_par[par], wq_t, bqT_t),
                                      (kT_par[par], wk_t, bkT_t)):
                    for p in range(2):
                        gs.append(lambda J=J, par=par, dst=dst, w_t=w_t,
                                  b_t=b_t, p=p:
                                  qk_group(J, par, dst, w_t, b_t, p, ceng))
                for r in range(4):
                    gs.append(lambda J=J, par=par, r=r:
                              v_group(J, par, r, ceng))
                return gs

            def wo_qsub(J, qsub, ceng=None):
                """W_O for one 128-row qsub of block J: 2 XBAR transposes +
                2 psum groups + copies + one merged prt row-block DMA."""
                for hp in range(2):
                    t_tz = ztpp.tile([128, 128], BF16, tag="ztp", name="tz")
                    eng = nc.sync if hp == 0 else nc.scalar
                    eng.dma_start_transpose(t_tz[:], zts[(J, hp, qsub)][:])
                    tz_stash[(J, qsub, hp)] = t_tz
                o_sb = osbp.tile([128, D], rdt, tag="o_sb")
                for n2 in range(2):
                    msl = slice(512 * n2, 512 * (n2 + 1))
                    oacc = psA.tile([128, 512], F32, tag="a_ps")
                    for hp in range(2):
                        nc.tensor.matmul(oacc[:], tz_stash[(J, qsub, hp)][:],
                                         wo_t[:, hp, msl],
                                         start=(hp == 0), stop=(hp == 1))
                    if bias:
                        # host pre-scales b_O by 1/GPC: every core adds
                        # bias/GPC so the ReduceScatter sum is exact.
                        nc.vector.tensor_tensor(o_sb[:, msl], oacc[:],
                                                bo_bc[:, msl], ADD)
                    else:
                        (ceng or nc.vector).tensor_copy(o_sb[:, msl], oacc[:])
                eng = nc.sync if qsub % 2 == 0 else nc.scalar
                if rs_rep:
                    eng.dma_start(
                        prtall[QB * qsub + 128 * J:QB * qsub + 128 * (J + 1),
                               :], o_sb[:])
                else:
                    eng.dma_start(
                        prt[J][128 * qsub:128 * (qsub + 1), :], o_sb[:])

            def wo_rs_out(J):
                if rs_rep:
                    if J != NJ - 1:
                        return
                    # one collective per rep over the concatenated blocks:
                    # prtall rows are (J, qsub) so the scatter slice for
                    # rank j is rows [512j:512j+512] = its stripes J=0..3
                    rs = dp.tile([QB, D], rdt, tag="rs", name="rs")
                    if collective:
                        nc.gpsimd.collective_compute(
                            "ReduceScatter", mybir.AluOpType.add,
                            replica_groups=RG,
                            ins=[prtall[:].opt()], outs=[rs[:].opt()])
                    else:
                        nc.sync.dma_start(rs[:], prtall[0:QB, :])
                    nc.scalar.dma_start(out_ext[:, :], rs[:])
                    return
                rs = dp.tile([QB // GPC, D], rdt, tag="rs", name="rs")
                if collective:
                    nc.gpsimd.collective_compute(
                        "ReduceScatter", mybir.AluOpType.add,
                        replica_groups=RG,
                        ins=[prt[J][:].opt()], outs=[rs[:].opt()])
                else:
                    nc.sync.dma_start(rs[:], prt[J][0:128, :])
                nc.scalar.dma_start(
                    out_ext[128 * J:128 * (J + 1), :], rs[:])

            def emit_S_exp(J, par, hp, c):
                qT, kT = qT_par[par], kT_par[par]
                dlt = c - 4 * J
                w0 = 128 * dlt if dlt >= 0 else 0
                csl = slice(KCH * c, KCH * (c + 1))
                qsl = slice(QB * J + w0, QB * (J + 1))
                lo, hi = slice(0, 64), slice(64, 128)
                s2 = psS.tile([KCH, 2, QB], F32, tag="s2")
                nc.tensor.matmul(s2[:, 0, w0:QB], kT[hp][lo, csl],
                                 qT[hp][lo, qsl], start=True, stop=True)
                nc.tensor.matmul(s2[:, 1, w0:QB], kT[hp][hi, csl],
                                 qT[hp][hi, qsl], start=True, stop=True)
                p2 = p2p.tile([KCH, 2, QB], BF16, tag="p2")
                nc.scalar.activation(p2[:, :, w0:QB], s2[:, :, w0:QB],
                                     EXP, scale=SCALE)
                if dlt >= 0:
                    nc.vector.tensor_mul(p2[:, :, w0:w0 + KCH],
                                         p2[:, :, w0:w0 + KCH], tri_m[:])
                return p2

            def emit_PV(J, par, hp, p2s, after_qsub=None):
                v_aug = va_par[par]
                for qsub in range(4):
                    zt = psZT.tile([KCH, 2, DH + 1], F32, tag="zt")
                    nch_q = 4 * J + qsub + 1
                    qo = 128 * qsub
                    for hh in range(2):
                        h = 2 * hp + hh
                        for c in range(nch_q):
                            nc.tensor.matmul(
                                zt[:, hh, :],
                                p2s[c][:, hh, qo:qo + 128],
                                v_aug[c][:, h, :],
                                start=(c == 0), stop=(c == nch_q - 1))
                    rsb = rsp.tile([KCH, 2, 1], F32, tag="rsb")
                    nc.vector.reciprocal(rsb[:], zt[:, :, DH:DH + 1])
                    zt_sb = ztsbp.tile([KCH, 2, DH], BF16, tag="ztsb")
                    for hh in range(2):
                        nc.vector.tensor_scalar(
                            zt_sb[:, hh, :], zt[:, hh, 0:DH],
                            rsb[:, hh, :], None, MUL)
                    zts[(J, hp, qsub)] = zt_sb
                    if after_qsub is not None:
                        after_qsub(qsub)

            for _rep in range(reps):
                par = _rep % npar
                if rs_rep:
                    prtall = dp.tile([GPC * QB, D], rdt, tag="prtall",
                                     name="prtall")
                for jb in range(NJ):
                    J = jb
                    nch = 4 * (J + 1)
                    prt[J] = dp.tile([QB, D], rdt, tag="prt", name="prt")
                    if _rep == 0 and J == 0:
                        # bootstrap: weights + rep-0 x/pos + QKV(0)
                        if not weights_loaded:
                            nc.sync.dma_start(wq_t[:], wqp[:, :])
                        emit_producers(par)
                        if not weights_loaded:
                            nc.scalar.dma_start(wk_t[:], wkp[:, :])
                            nc.sync.dma_start(wv_t[:], wvp[:, :])
                            if ag:
                                nc.scalar.dma_start(wo_t[:], wof[:, :])
                                nc.scalar.dma_start(sel_t[:], sel[:, :])
                            else:
                                nc.scalar.dma_start(wo_t[:], wop[:, :])
                            nc.scalar.dma_start(tri_m[:, :, :], masks[:, :])
                            nc.scalar.dma_start(bqT_t[:], bqT[:, :])
                            nc.scalar.dma_start(bkT_t[:], bkT[:, :])
                            nc.scalar.dma_start(bv_t[:], bv[:, :])
                            nc.scalar.dma_start(bo_t[:], bo[:, :])
                            weights_loaded = True
                        for g in qkv_groups(0, par):
                            g()

                    # filler: PE work injectable between score chunks
                    filler = []
                    if J >= 1:
                        Jw = J - 1
                        woc = None
                        for q in range(4):
                            filler.append(lambda Jw=Jw, q=q, woc=woc:
                                          wo_qsub(Jw, q, woc))
                        filler.append(lambda Jw=Jw: wo_rs_out(Jw))
                    elif _rep >= 1:
                        for q in range(4):
                            filler.append(lambda q=q: wo_qsub(3, q))
                        filler.append(lambda: wo_rs_out(3))
                    if J + 1 < NJ:
                        qc = None
                        filler += qkv_groups(J + 1, par, qc)
                    elif _rep + 1 < reps:
                        emit_producers((_rep + 1) % npar)
                        filler += qkv_groups(0, (_rep + 1) % npar)

                    last_block = (_rep == reps - 1 and J == NJ - 1)
                    slots = 2 * nch
                    for hp in range(2):
                        p2s = []
                        for c in range(nch):
                            p2s.append(emit_S_exp(J, par, hp, c))
                            take = (math.ceil(len(filler) / slots)
                                    if slots > 1 else len(filler))
                            for _ in range(take):
                                filler.pop(0)()
                            slots -= 1
                        if last_block and hp == 1:
                            # tail: fold W_O(3) into PV(3, hp1) per qsub
                            def _tail(qsub):
                                wo_qsub(3, qsub)
                                if qsub == 3:
                                    wo_rs_out(3)
                            emit_PV(J, par, hp, p2s, after_qsub=_tail)
                        else:
                            emit_PV(J, par, hp, p2s)
    nc.compile()
    return nc


def _make_masks():
    # [128, 2*128] causal triangle duplicated for the head-pair layout:
    # tri[k, j] = 1 if k <= j (the diagonal band of every 128-key chunk
    # relative to its causal column start)
    k = np.arange(KCH)[:, None]
    j = np.arange(KCH)[None, :]
    tri = (k <= j).astype(np.float32)
    return np.ascontiguousarray(np.concatenate([tri, tri], axis=1))


def _prechunk(w):
    """[1024, C] -> [128, DCH*C] with kc-major free layout."""
    c = w.shape[1]
    return np.ascontiguousarray(
        w.reshape(DCH, 128, c).transpose(1, 0, 2).reshape(128, DCH * c))


def make_in_maps(x, pos_embed, W_Q, b_Q, W_K, b_K, W_V, b_V, W_O, b_O):
    import ml_dtypes
    bf = ml_dtypes.bfloat16
    x = np.asarray(x, np.float32)
    pos_embed = np.asarray(pos_embed, np.float32)
    W_Q = np.asarray(W_Q, np.float32)
    W_K = np.asarray(W_K, np.float32)
    W_V = np.asarray(W_V, np.float32)
    W_O = np.asarray(W_O, np.float32)
    b_Q = np.asarray(b_Q, np.float32)
    b_K = np.asarray(b_K, np.float32)
    b_V = np.asarray(b_V, np.float32)
    b_O = np.asarray(b_O, np.float32)
    masks = _make_masks().astype(bf)
    in_maps = []
    for c in range(N_CORES):
        g, j = divmod(c, GPC)
        hs = slice(HPC * j, HPC * (j + 1))
        wo_pair = np.ascontiguousarray(
            W_O[hs].reshape(2, 128, D).transpose(1, 0, 2).reshape(128, 2 * D))
        in_maps.append({
            "xT": np.ascontiguousarray(x[g].T).astype(bf),
            "posT": np.ascontiguousarray(pos_embed[g].T).astype(bf),
            "wqp": _prechunk(
                W_Q[hs].transpose(1, 0, 2).reshape(D, HPC * DH)).astype(bf),
            "wkp": _prechunk(
                W_K[hs].transpose(1, 0, 2).reshape(D, HPC * DH)).astype(bf),
            "wvp": _prechunk(
                W_V[hs].transpose(1, 0, 2).reshape(D, HPC * DH)).astype(bf),
            "wop": wo_pair.astype(bf),
            "bqT": np.ascontiguousarray(
                b_Q[hs].reshape(2, KCH).T).astype(bf),
            "bkT": np.ascontiguousarray(
                b_K[hs].reshape(2, KCH).T).astype(bf),
            "bv": np.ascontiguousarray(
                b_V[hs].reshape(1, HPC * DH)).astype(bf),
            "bo": np.ascontiguousarray(
                (b_O / GPC).reshape(1, D)).astype(bf),
            "masks": masks,
        })
    return in_maps


def assemble_out(results):
    out = np.empty((B, S, D), np.float32)
    for c in range(N_CORES):
        g, j = divmod(c, GPC)
        o = results[c]["out"].astype(np.float32).reshape(NJ, 128, D)
        for J in range(NJ):
            out[g, QB * J + 128 * j:QB * J + 128 * (j + 1), :] = o[J]
    return out


_BUILT = {}


def get_built(reps: int = 1, bias: bool = True, collective: bool = True,
              rs_f32: bool = False, rs_rep: bool = False):
    key = (reps, bias, collective, rs_f32, rs_rep)
    if key not in _BUILT:
        _BUILT[key] = build_nc(reps, collective=collective, bias=bias,
                               rs_f32=rs_f32, rs_rep=rs_rep)
    return _BUILT[key]


def kernel(**inputs) -> np.ndarray:
    use_bias = any(
        np.any(np.asarray(inputs[k])) for k in ("b_Q", "b_K", "b_V", "b_O"))
    nc = get_built(1, bias=bool(use_bias))
    in_maps = make_in_maps(**inputs)
    res = run_bass_kernel_spmd(nc, in_maps, list(range(N_CORES)))
    return assemble_out(res.results)
